# revision 17
# baseline (speedup 1.0000x reference)
"""Trainium2 Bass kernel for nn_Attention_CA (sparse_attention).

Reference computation (NUM_HEADS=8):
    x_pool = avgpool4(kv)                  # [b, 96, 4096]
    q = l2norm(Q.reshape(b, 8, 48, 65536)) # over last axis
    k = v = l2norm(x_pool.reshape(b, 8, 12, 4096))
    k, v tiled 16x along length -> 65536
    attn = softmax(q @ k^T)                # [b, 8, 48, 12]
    out  = attn @ v                        # [b, 8, 48, 65536]
    y    = W_proj @ out                    # 1x1 conv over channels

Algebraic structure exploited:
  * q @ tile(k,16)^T == fold16(q) @ k^T where fold16 sums the 16 length-4096
    chunks of each q row.  The q l2-normalisation is a per-row scalar, so it
    becomes a per-row scale of the logits (a softmax temperature).
  * attn @ tile(v,16) is 16x periodic along the length dim, and so is the
    1x1 projection of it.  The device therefore only produces y_small
    [2, 384, 4096]; the host materialises the full [2, 384, 256, 256] output
    by tiling (exact, not an approximation).

Sharding over 8 cores: core i owns batch b=i//4 and heads {2a, 2a+1}
(a = i%4), i.e. 96 q-channel rows, 96 raw kv rows (-> 24 pooled rows) and 96
output channels of W_proj.  The only cross-core exchange is an AllGather of
the per-core attention outputs [96, 4096] within each batch group of 4
cores, ahead of the channel-contracting 1x1 projection.
"""

import numpy as np

NUM_HEADS = 8
B, C, H, W = 2, 384, 256, 256
HW = H * W           # 65536
L = 4096             # kv length == pooled row length
J = HW // L          # 16 fold chunks
CQ = C // NUM_HEADS  # 48 q rows per head
ROWS = 96            # q rows per core (2 heads)
KR = 24              # pooled kv rows per core (2 heads x 12)
NCORES = 8
GROUP = 4            # cores per batch
EPS = 1e-12

_CACHE = {}


def _build():
    import os as _os
    import concourse.bacc as bacc
    import concourse.mybir as mybir
    from concourse.tile import TileContext

    STAGE = int(_os.environ.get("KERNEL_STAGE", "4"))
    f32 = mybir.dt.float32
    Alu = mybir.AluOpType
    Act = mybir.ActivationFunctionType

    nc = bacc.Bacc(num_devices=NCORES)

    q_in = nc.dram_tensor("q", [ROWS, J, L], f32, kind="ExternalInput")
    kv_in = nc.dram_tensor("kv", [ROWS, L], f32, kind="ExternalInput")
    w_in = nc.dram_tensor("w", [ROWS, C], f32, kind="ExternalInput")
    y_out = nc.dram_tensor("y", [ROWS, L], f32, kind="ExternalOutput")

    # constants baked into the NEFF
    ident_np = np.eye(128, dtype=np.float32)
    poolmat_np = np.zeros((ROWS, KR), dtype=np.float32)
    for k in range(KR):
        poolmat_np[4 * k:4 * k + 4, k] = 0.25
    # block-diagonal head mask: head0 rows see cols 0:12, head1 rows 12:24
    mask_np = np.zeros((ROWS, KR), dtype=np.float32)
    mask_np[:CQ, :12] = 1.0
    mask_np[CQ:, 12:] = 1.0
    ident_dram = nc.inline_tensor(ident_np, name="ident")
    poolmat_dram = nc.inline_tensor(poolmat_np, name="poolmat")
    mask_dram = nc.inline_tensor(mask_np, name="mask")

    # collective bounce buffers
    so_dram = nc.dram_tensor("so_local", [ROWS * L], f32)
    gath_dram = nc.dram_tensor("so_gath", [GROUP * ROWS * L], f32)

    NT = L // 128        # 32 column tiles of the folded q
    NMM = L // 512       # 8 matmul column chunks

    with TileContext(nc) as tc:
        with (
            tc.tile_pool(name="big", bufs=3) as big_pool,
            tc.tile_pool(name="persist", bufs=1) as persist,
            tc.tile_pool(name="small", bufs=2) as small,
        ):
            with tc.tile_pool(name="psum", bufs=2, space="PSUM") as psum:
                ident = persist.tile([128, 128], f32)
                nc.sync.dma_start(out=ident, in_=ident_dram[:, :])
                poolmat = persist.tile([ROWS, KR], f32)
                nc.sync.dma_start(out=poolmat, in_=poolmat_dram[:, :])

                # ---- kv: pool -> l2-normalise (independent of Q) ----
                kv_sb = big_pool.tile([ROWS, L], f32, tag="kv", bufs=1)
                nc.sync.dma_start(out=kv_sb, in_=kv_in[:, :])
                kn = persist.tile([KR, L], f32)
                for n in range(NMM):
                    ppool = psum.tile([KR, 512], f32, tag="mm512")
                    nc.tensor.matmul(ppool, lhsT=poolmat,
                                     rhs=kv_sb[:, n * 512:(n + 1) * 512],
                                     start=True, stop=True)
                    nc.scalar.copy(kn[:, n * 512:(n + 1) * 512], ppool)
                ksq = small.tile([KR, 1], f32)
                ksc = big_pool.tile([KR, L], f32, tag="sqscr", bufs=1)
                nc.scalar.activation(ksc, kn, Act.Square, accum_out=ksq)
                knrm = small.tile([KR, 1], f32)
                nc.scalar.sqrt(knrm, ksq)
                nc.vector.tensor_scalar_max(knrm, knrm, EPS)
                kinv = small.tile([KR, 1], f32)
                nc.vector.reciprocal(kinv, knrm)
                nc.vector.tensor_scalar_mul(kn, kn, kinv)

                # ---- W slice, transposed into 4 chunks of K=96 (early) ----
                w_sb = persist.tile([ROWS, C], f32)
                nc.sync.dma_start(out=w_sb, in_=w_in[:, :])
                wT = persist.tile([ROWS, GROUP, ROWS], f32)
                for g in range(GROUP):
                    pw = psum.tile([ROWS, ROWS], f32, tag="tp")
                    nc.tensor.transpose(pw, w_sb[:, g * ROWS:(g + 1) * ROWS],
                                        ident[:ROWS, :ROWS])
                    nc.scalar.copy(wT[:, g, :], pw)

                # transpose kn -> knT [128, NT, KR]
                knT = persist.tile([128, NT, KR], f32)
                for t in range(NT):
                    pt = psum.tile([128, KR], f32, tag="tp")
                    nc.tensor.transpose(pt, kn[:, t * 128:(t + 1) * 128],
                                        ident[:KR, :KR])
                    nc.scalar.copy(knT[:, t, :], pt)

                # ---- Q: fold 16 chunks + sum of squares ----
                acc = persist.tile([ROWS, L], f32)
                sqparts = persist.tile([ROWS, J], f32)
                for j in range(J):
                    chunk = big_pool.tile([ROWS, L], f32, tag="chunk")
                    nc.sync.dma_start(out=chunk, in_=q_in[:, j, :])
                    if j == 0:
                        nc.vector.tensor_copy(acc, chunk)
                    else:
                        nc.vector.tensor_add(acc, acc, chunk)
                    sqscr = big_pool.tile([ROWS, L], f32, tag="sqscr", bufs=1)
                    nc.scalar.activation(sqscr, chunk, Act.Square,
                                         accum_out=sqparts[:, j:j + 1])

                sumsq = small.tile([ROWS, 1], f32)
                nc.vector.reduce_sum(sumsq, sqparts, axis=mybir.AxisListType.X)
                qnrm = small.tile([ROWS, 1], f32)
                nc.scalar.sqrt(qnrm, sumsq)
                nc.vector.tensor_scalar_max(qnrm, qnrm, EPS)
                qinv = small.tile([ROWS, 1], f32)
                nc.vector.reciprocal(qinv, qnrm)

                # transpose acc -> qfT [128, NT, ROWS]
                qfT = persist.tile([128, NT, ROWS], f32)
                for t in range(NT):
                    ptq = psum.tile([128, ROWS], f32, tag="tp")
                    nc.tensor.transpose(ptq, acc[:, t * 128:(t + 1) * 128],
                                        ident[:ROWS, :ROWS])
                    nc.scalar.copy(qfT[:, t, :], ptq)

                # ---- attention logits for both heads in one matmul chain ---
                # out[96, 24]: block diag [48x12 | 48x12] is valid, rest junk
                pattn = psum.tile([ROWS, KR], f32, tag="attn", bufs=1)
                for t in range(NT):
                    nc.tensor.matmul(pattn, lhsT=qfT[:, t, :], rhs=knT[:, t, :],
                                     start=(t == 0), stop=(t == NT - 1))

                # ---- softmax (no max-subtraction: |logits| <= 4) ----
                # Scale+exp the full [96, 24] (off-block junk is bounded),
                # then mask block-diagonally while row-summing in one DVE op.
                mask_sb = persist.tile([ROWS, KR], f32)
                nc.sync.dma_start(out=mask_sb, in_=mask_dram[:, :])
                e_sb = small.tile([ROWS, KR], f32)
                nc.vector.tensor_scalar(e_sb, pattn, qinv, None, Alu.mult)
                nc.scalar.activation(e_sb, e_sb, Act.Exp)
                p_sb = small.tile([ROWS, KR], f32)
                nc.vector.tensor_mul(p_sb, e_sb, mask_sb)
                esum = small.tile([ROWS, 1], f32)
                nc.vector.reduce_sum(esum, p_sb, axis=mybir.AxisListType.X)
                einv = small.tile([ROWS, 1], f32)
                nc.vector.reciprocal(einv, esum)

                # one PE transpose yields block-diagonal pT [24, 96]
                pT = small.tile([KR, ROWS], f32)
                ptp = psum.tile([KR, ROWS], f32, tag="tp")
                nc.tensor.transpose(ptp, p_sb, ident[:ROWS, :ROWS])
                nc.scalar.copy(pT, ptp)

                # ---- small_out = softmax(p) @ kn (both heads at once) ----
                # the 1/sum(exp) row scale is applied on the PSUM->SBUF copy
                so_sb = persist.tile([ROWS, L], f32)
                for n in range(NMM):
                    pso = psum.tile([ROWS, 512], f32, tag="mm512")
                    nc.tensor.matmul(pso, lhsT=pT,
                                     rhs=kn[:, n * 512:(n + 1) * 512],
                                     start=True, stop=True)
                    nc.scalar.activation(so_sb[:, n * 512:(n + 1) * 512], pso,
                                         Act.Copy, scale=einv)

                # ---- AllGather within each batch group of 4 cores ----
                nc.sync.dma_start(
                    out=so_dram[:].rearrange("(p f) -> p f", p=ROWS),
                    in_=so_sb)
                import os as _os
                if _os.environ.get("KERNEL_NO_CC"):
                    gfold = gath_dram[:].rearrange("(g x) -> g x", g=GROUP)
                    for g in range(GROUP):
                        nc.sync.dma_start(out=gfold[g], in_=so_dram[:])
                else:
                    nc.gpsimd.collective_compute(
                        "AllGather", Alu.bypass,
                        replica_groups=[[0, 1, 2, 3], [4, 5, 6, 7]],
                        ins=[so_dram[:]],
                        outs=[gath_dram[:]],
                    )

            # ---- projection: y = W[o_slice, :] @ gathered ----
            # first PSUM pool released; use all 8 banks as accumulators
            with tc.tile_pool(name="psum_y", bufs=1, space="PSUM") as psum_y:
                gath_ap = gath_dram[:].rearrange("(g p f) -> g p f",
                                                 g=GROUP, p=ROWS)
                py = [psum_y.tile([ROWS, 512], f32, tag=f"y{n}",
                                  name=f"py{n}") for n in range(NMM)]
                for g in range(GROUP):
                    gt = big_pool.tile([ROWS, L], f32, tag="gath", bufs=2)
                    nc.sync.dma_start(out=gt, in_=gath_ap[g, :, :])
                    for n in range(NMM):
                        nc.tensor.matmul(py[n], lhsT=wT[:, g, :],
                                         rhs=gt[:, n * 512:(n + 1) * 512],
                                         start=(g == 0), stop=(g == GROUP - 1))
                y_sb = persist.tile([ROWS, L], f32)
                for n in range(NMM):
                    nc.scalar.copy(y_sb[:, n * 512:(n + 1) * 512], py[n])
                nc.sync.dma_start(out=y_out[:, :], in_=y_sb)

    if not nc.is_finalized():
        nc.finalize()
    return nc


def _get_nc():
    if "nc" not in _CACHE:
        _CACHE["nc"] = _build()
    return _CACHE["nc"]


def kernel(Q, kv, W_proj, _trace=False):
    from concourse.bass_utils import run_bass_kernel_spmd

    Q = np.ascontiguousarray(Q, dtype=np.float32)
    kv = np.ascontiguousarray(kv, dtype=np.float32)
    W_proj = np.ascontiguousarray(W_proj, dtype=np.float32)

    Qr = Q.reshape(B * C, J, L)
    in_maps = []
    for i in range(NCORES):
        b, a = divmod(i, GROUP)
        sl = slice(96 * a, 96 * a + 96)
        in_maps.append({
            "q": np.ascontiguousarray(Qr[b * C + 96 * a: b * C + 96 * a + 96]),
            "kv": np.ascontiguousarray(kv[b, sl]),
            "w": np.ascontiguousarray(W_proj[sl]),
        })

    nc = _get_nc()
    res = run_bass_kernel_spmd(nc, in_maps, core_ids=list(range(NCORES)),
                               trace=_trace)
    _CACHE["last_results"] = res

    y_small = np.empty((B, C, L), np.float32)
    for i in range(NCORES):
        b, a = divmod(i, GROUP)
        y_small[b, 96 * a: 96 * a + 96] = res.results[i]["y"].reshape(ROWS, L)

    out = np.broadcast_to(y_small[:, :, None, :], (B, C, J, L))
    return np.ascontiguousarray(out).reshape(B, C, H, W)


# revision 29
# speedup vs baseline: 1.3941x; 1.3941x over previous
"""Trainium2 Bass kernel for nn_Attention_CA (sparse_attention).

Reference computation (NUM_HEADS=8):
    x_pool = avgpool4(kv)                  # [b, 96, 4096]
    q = l2norm(Q.reshape(b, 8, 48, 65536)) # over last axis
    k = v = l2norm(x_pool.reshape(b, 8, 12, 4096))
    k, v tiled 16x along length -> 65536
    attn = softmax(q @ k^T)                # [b, 8, 48, 12]
    out  = attn @ v                        # [b, 8, 48, 65536]
    y    = W_proj @ out                    # 1x1 conv over channels

Algebraic structure exploited:
  * q @ tile(k,16)^T == fold16(q) @ k^T where fold16 sums the 16 length-4096
    chunks of each q row.  The q l2-normalisation is a per-row scalar, so it
    becomes a per-row scale of the logits (a softmax temperature).
  * attn @ tile(v,16) is 16x periodic along the length dim, and so is the
    1x1 projection of it.  The device therefore only produces y_small
    [2, 384, 4096]; the host materialises the full [2, 384, 256, 256] output
    by tiling (exact, not an approximation).

Sharding over 8 cores: core i owns batch b=i//4 and heads {2a, 2a+1}
(a = i%4), i.e. 96 q-channel rows, 96 raw kv rows (-> 24 pooled rows) and 96
output channels of W_proj.  The only cross-core exchange is an AllGather of
the per-core attention outputs [96, 4096] within each batch group of 4
cores, ahead of the channel-contracting 1x1 projection.
"""

import numpy as np

NUM_HEADS = 8
B, C, H, W = 2, 384, 256, 256
HW = H * W           # 65536
L = 4096             # kv length == pooled row length
J = HW // L          # 16 fold chunks
CQ = C // NUM_HEADS  # 48 q rows per head
ROWS = 96            # q rows per core (2 heads)
KR = 24              # pooled kv rows per core (2 heads x 12)
NCORES = 8
GROUP = 4            # cores per batch
EPS = 1e-12

_CACHE = {}


def _build():
    import os as _os
    import concourse.bacc as bacc
    import concourse.mybir as mybir
    from concourse.tile import TileContext

    STAGE = int(_os.environ.get("KERNEL_STAGE", "4"))
    f32 = mybir.dt.float32
    Alu = mybir.AluOpType
    Act = mybir.ActivationFunctionType

    nc = bacc.Bacc(num_devices=NCORES)

    MQ = L // NCORES     # 512: m-slice each core projects
    q_in = nc.dram_tensor("q", [ROWS, J, L], f32, kind="ExternalInput")
    kv_in = nc.dram_tensor("kv", [ROWS, L], f32, kind="ExternalInput")
    w_in = nc.dram_tensor("w", [C, C], f32, kind="ExternalInput")
    y_out = nc.dram_tensor("y", [B, C, MQ], f32, kind="ExternalOutput")

    # constants baked into the NEFF
    ident_np = np.eye(128, dtype=np.float32)
    poolmat_np = np.zeros((ROWS, KR), dtype=np.float32)
    for k in range(KR):
        poolmat_np[4 * k:4 * k + 4, k] = 0.25
    # block-diagonal head mask: head0 rows see cols 0:12, head1 rows 12:24
    mask_np = np.zeros((ROWS, KR), dtype=np.float32)
    mask_np[:CQ, :12] = 1.0
    mask_np[CQ:, 12:] = 1.0
    ident_dram = nc.inline_tensor(ident_np, name="ident")
    poolmat_dram = nc.inline_tensor(poolmat_np, name="poolmat")
    mask_dram = nc.inline_tensor(mask_np, name="mask")

    # collective bounce buffers (8-core AllToAll: m-eighths <-> channels)
    so_dram = nc.dram_tensor("so_local", [NCORES * ROWS * MQ], f32)
    a2a_dram = nc.dram_tensor("so_a2a", [NCORES * ROWS * MQ], f32)

    NT = L // 128        # 32 column tiles of the folded q
    NMM = L // 512       # 8 matmul column chunks

    with TileContext(nc) as tc:
        with (
            tc.tile_pool(name="big", bufs=3) as big_pool,
            tc.tile_pool(name="persist", bufs=1) as persist,
            tc.tile_pool(name="small", bufs=2) as small,
        ):
            with tc.tile_pool(name="psum", bufs=2, space="PSUM") as psum:
                ident = persist.tile([128, 128], f32)
                nc.sync.dma_start(out=ident, in_=ident_dram[:, :])
                poolmat = persist.tile([ROWS, KR], f32)
                nc.sync.dma_start(out=poolmat, in_=poolmat_dram[:, :])

                # ---- kv: pool -> l2-normalise (independent of Q) ----
                kv_sb = big_pool.tile([ROWS, L], f32, tag="kv", bufs=1)
                nc.sync.dma_start(out=kv_sb, in_=kv_in[:, :])
                kn = persist.tile([KR, L], f32)
                for n in range(NMM):
                    ppool = psum.tile([KR, 512], f32, tag="mm512")
                    nc.tensor.matmul(ppool, lhsT=poolmat,
                                     rhs=kv_sb[:, n * 512:(n + 1) * 512],
                                     start=True, stop=True)
                    nc.scalar.copy(kn[:, n * 512:(n + 1) * 512], ppool)
                ksq = small.tile([KR, 1], f32)
                ksc = big_pool.tile([KR, L], f32, tag="sqscr", bufs=1)
                nc.scalar.activation(ksc, kn, Act.Square, accum_out=ksq)
                knrm = small.tile([KR, 1], f32)
                nc.scalar.sqrt(knrm, ksq)
                nc.vector.tensor_scalar_max(knrm, knrm, EPS)
                kinv = small.tile([KR, 1], f32)
                nc.vector.reciprocal(kinv, knrm)
                nc.vector.tensor_scalar_mul(kn, kn, kinv)

                # ---- full W, transposed: K-chunks of 96, O-blocks of 128 --
                NB = C // 128   # 3 output-row blocks
                NK = GROUP      # 4 channel chunks of 96 (align a2a blocks)
                w_sb = persist.tile([128, NB, C], f32)
                nc.sync.dma_start(
                    out=w_sb,
                    in_=w_in[:, :].rearrange("(b p) c -> p b c", p=128))
                wT = persist.tile([ROWS, NK, NB, 128], f32)
                for kc in range(NK):
                    for ob in range(NB):
                        pw = psum.tile([ROWS, 128], f32, tag="tp")
                        nc.tensor.transpose(
                            pw, w_sb[:, ob, kc * ROWS:(kc + 1) * ROWS],
                            ident)
                        nc.scalar.copy(wT[:, kc, ob, :], pw)

                # transpose kn -> knT [128, NT, KR]
                knT = persist.tile([128, NT, KR], f32)
                for t in range(NT):
                    pt = psum.tile([128, KR], f32, tag="tp")
                    nc.tensor.transpose(pt, kn[:, t * 128:(t + 1) * 128],
                                        ident[:KR, :KR])
                    nc.scalar.copy(knT[:, t, :], pt)

                # ---- Q: fold 16 chunks + sum of squares ----
                acc = persist.tile([ROWS, L], f32)
                sqparts = persist.tile([ROWS, J], f32)
                for j in range(J):
                    chunk = big_pool.tile([ROWS, L], f32, tag="chunk", bufs=4)
                    nc.sync.dma_start(out=chunk, in_=q_in[:, j, :])
                    if j == 0:
                        nc.vector.tensor_copy(acc, chunk)
                    else:
                        nc.vector.tensor_add(acc, acc, chunk)
                    sqscr = big_pool.tile([ROWS, L], f32, tag="sqscr", bufs=1)
                    nc.scalar.activation(sqscr, chunk, Act.Square,
                                         accum_out=sqparts[:, j:j + 1])

                sumsq = small.tile([ROWS, 1], f32)
                nc.vector.reduce_sum(sumsq, sqparts, axis=mybir.AxisListType.X)
                qnrm = small.tile([ROWS, 1], f32)
                nc.scalar.sqrt(qnrm, sumsq)
                nc.vector.tensor_scalar_max(qnrm, qnrm, EPS)
                qinv = small.tile([ROWS, 1], f32)
                nc.vector.reciprocal(qinv, qnrm)

                # transpose acc -> qfT [128, NT, ROWS]
                qfT = persist.tile([128, NT, ROWS], f32)
                for t in range(NT):
                    ptq = psum.tile([128, ROWS], f32, tag="tp")
                    nc.tensor.transpose(ptq, acc[:, t * 128:(t + 1) * 128],
                                        ident[:ROWS, :ROWS])
                    nc.scalar.copy(qfT[:, t, :], ptq)

                # ---- attention logits for both heads in one matmul chain ---
                # out[96, 24]: block diag [48x12 | 48x12] is valid, rest junk
                pattn = psum.tile([ROWS, KR], f32, tag="attn", bufs=1)
                for t in range(NT):
                    nc.tensor.matmul(pattn, lhsT=qfT[:, t, :], rhs=knT[:, t, :],
                                     start=(t == 0), stop=(t == NT - 1))

                # ---- softmax (no max-subtraction: |logits| <= 4) ----
                # Scale+exp the full [96, 24] (off-block junk is bounded),
                # then mask block-diagonally while row-summing in one DVE op.
                mask_sb = persist.tile([ROWS, KR], f32)
                nc.sync.dma_start(out=mask_sb, in_=mask_dram[:, :])
                e_sb = small.tile([ROWS, KR], f32)
                nc.vector.tensor_scalar(e_sb, pattn, qinv, None, Alu.mult)
                nc.scalar.activation(e_sb, e_sb, Act.Exp)
                p_sb = small.tile([ROWS, KR], f32)
                nc.vector.tensor_mul(p_sb, e_sb, mask_sb)
                esum = small.tile([ROWS, 1], f32)
                nc.vector.reduce_sum(esum, p_sb, axis=mybir.AxisListType.X)
                einv = small.tile([ROWS, 1], f32)
                nc.vector.reciprocal(einv, esum)

                # one PE transpose yields block-diagonal pT [24, 96]
                pT = small.tile([KR, ROWS], f32)
                ptp = psum.tile([KR, ROWS], f32, tag="tp")
                nc.tensor.transpose(ptp, p_sb, ident[:ROWS, :ROWS])
                nc.scalar.copy(pT, ptp)

                # ---- small_out = softmax(p) @ kn (both heads at once) ----
                # the 1/sum(exp) row scale is applied on the PSUM->SBUF copy
                so_sb = persist.tile([ROWS, L], f32)
                for n in range(NMM):
                    pso = psum.tile([ROWS, 512], f32, tag="mm512")
                    nc.tensor.matmul(pso, lhsT=pT,
                                     rhs=kn[:, n * 512:(n + 1) * 512],
                                     start=True, stop=True)
                    nc.scalar.activation(so_sb[:, n * 512:(n + 1) * 512], pso,
                                         Act.Copy, scale=einv)

                # ---- 8-core AllToAll: shard r = so[:, 512r:512r+512] ----
                # received block g = [96 channel rows of batch g//4,
                # channel block g%4, my m-eighth]
                nc.sync.dma_start(
                    out=so_dram[:].rearrange("(g p m) -> p g m",
                                             g=NCORES, p=ROWS),
                    in_=so_sb.rearrange("p (g m) -> p g m", g=NCORES))
                nc.gpsimd.collective_compute(
                    "AllToAll", Alu.bypass,
                    replica_groups=[[0, 1, 2, 3, 4, 5, 6, 7]],
                    ins=[so_dram[:]],
                    outs=[a2a_dram[:]],
                )

            # ---- projection: y[b, :, my m-eighth] = W @ so_all[b] ----
            # first PSUM pool released; 6 banks as accumulators
            with tc.tile_pool(name="psum_y", bufs=1, space="PSUM") as psum_y:
                a2a_ap = a2a_dram[:].rearrange("(g p m) -> g p m",
                                               g=NCORES, p=ROWS)
                py = [[psum_y.tile([128, MQ], f32, tag=f"y{b}{ob}",
                                   name=f"py{b}{ob}") for ob in range(NB)]
                      for b in range(B)]
                for b in range(B):
                    for kc in range(NK):
                        gt = big_pool.tile([ROWS, MQ], f32, tag="gath",
                                           bufs=2)
                        nc.sync.dma_start(out=gt,
                                          in_=a2a_ap[GROUP * b + kc, :, :])
                        for ob in range(NB):
                            nc.tensor.matmul(
                                py[b][ob], lhsT=wT[:, kc, ob, :], rhs=gt,
                                start=(kc == 0), stop=(kc == NK - 1))
                for b in range(B):
                    y_ap = y_out[b, :, :].rearrange("(ob p) m -> p ob m",
                                                    p=128)
                    for ob in range(NB):
                        y_sb = small.tile([128, MQ], f32, tag="ysb")
                        nc.scalar.copy(y_sb, py[b][ob])
                        nc.sync.dma_start(out=y_ap[:, ob, :], in_=y_sb)

    if not nc.is_finalized():
        nc.finalize()
    return nc


def _get_nc():
    if "nc" not in _CACHE:
        _CACHE["nc"] = _build()
    return _CACHE["nc"]


def kernel(Q, kv, W_proj, _trace=False):
    from concourse.bass_utils import run_bass_kernel_spmd

    Q = np.ascontiguousarray(Q, dtype=np.float32)
    kv = np.ascontiguousarray(kv, dtype=np.float32)
    W_proj = np.ascontiguousarray(W_proj, dtype=np.float32)

    Qr = Q.reshape(B * C, J, L)
    in_maps = []
    for i in range(NCORES):
        b, a = divmod(i, GROUP)
        sl = slice(96 * a, 96 * a + 96)
        in_maps.append({
            "q": np.ascontiguousarray(Qr[b * C + 96 * a: b * C + 96 * a + 96]),
            "kv": np.ascontiguousarray(kv[b, sl]),
            "w": W_proj,
        })

    nc = _get_nc()
    res = run_bass_kernel_spmd(nc, in_maps, core_ids=list(range(NCORES)),
                               trace=_trace)
    _CACHE["last_results"] = res

    MQ = L // NCORES
    y_small = np.empty((B, C, L), np.float32)
    for i in range(NCORES):
        y_small[:, :, MQ * i: MQ * (i + 1)] = res.results[i]["y"]

    out = np.broadcast_to(y_small[:, :, None, :], (B, C, J, L))
    return np.ascontiguousarray(out).reshape(B, C, H, W)


# revision 36
# speedup vs baseline: 1.5663x; 1.1235x over previous
"""Trainium2 Bass kernel for nn_Attention_CA (sparse_attention).

Reference computation (NUM_HEADS=8):
    x_pool = avgpool4(kv)                  # [b, 96, 4096]
    q = l2norm(Q.reshape(b, 8, 48, 65536)) # over last axis
    k = v = l2norm(x_pool.reshape(b, 8, 12, 4096))
    k, v tiled 16x along length -> 65536
    attn = softmax(q @ k^T)                # [b, 8, 48, 12]
    out  = attn @ v                        # [b, 8, 48, 65536]
    y    = W_proj @ out                    # 1x1 conv over channels

Algebraic structure exploited:
  * q @ tile(k,16)^T == fold16(q) @ k^T where fold16 sums the 16 length-4096
    chunks of each q row.  The q l2-normalisation is a per-row scalar, so it
    becomes a per-row scale of the logits (a softmax temperature).
  * attn @ tile(v,16) is 16x periodic along the length dim, and so is the
    1x1 projection of it.  The device therefore only produces y_small
    [2, 384, 4096]; the host materialises the full [2, 384, 256, 256] output
    by tiling (exact, not an approximation).

Sharding over 8 cores: core i owns batch b=i//4 and heads {2a, 2a+1}
(a = i%4), i.e. 96 q-channel rows, 96 raw kv rows (-> 24 pooled rows) and 96
output channels of W_proj.  The only cross-core exchange is an AllGather of
the per-core attention outputs [96, 4096] within each batch group of 4
cores, ahead of the channel-contracting 1x1 projection.
"""

import numpy as np

NUM_HEADS = 8
B, C, H, W = 2, 384, 256, 256
HW = H * W           # 65536
L = 4096             # kv length == pooled row length
J = HW // L          # 16 fold chunks
CQ = C // NUM_HEADS  # 48 q rows per head
ROWS = 96            # q rows per core (2 heads)
KR = 24              # pooled kv rows per core (2 heads x 12)
NCORES = 8
GROUP = 4            # cores per batch
EPS = 1e-12

_CACHE = {}


def _build():
    import os as _os
    import concourse.bacc as bacc
    import concourse.mybir as mybir
    from concourse.tile import TileContext

    STAGE = int(_os.environ.get("KERNEL_STAGE", "4"))
    f32 = mybir.dt.float32
    bf16 = mybir.dt.bfloat16
    Alu = mybir.AluOpType
    Act = mybir.ActivationFunctionType

    nc = bacc.Bacc(num_devices=NCORES)

    MQ = L // NCORES     # 512: m-slice each core projects
    q_in = nc.dram_tensor("q", [ROWS, J, L], f32, kind="ExternalInput")
    kv_in = nc.dram_tensor("kv", [ROWS, L], f32, kind="ExternalInput")
    w_in = nc.dram_tensor("w", [C, C], f32, kind="ExternalInput")
    y_out = nc.dram_tensor("y", [B, C, MQ], f32, kind="ExternalOutput")

    # constants baked into the NEFF
    ident_np = np.eye(128, dtype=np.float32)
    poolmat_np = np.zeros((ROWS, KR), dtype=np.float32)
    for k in range(KR):
        poolmat_np[4 * k:4 * k + 4, k] = 0.25
    # block-diagonal head mask: head0 rows see cols 0:12, head1 rows 12:24
    mask_np = np.zeros((ROWS, KR), dtype=np.float32)
    mask_np[:CQ, :12] = 1.0
    mask_np[CQ:, 12:] = 1.0
    ident_dram = nc.inline_tensor(ident_np, name="ident")
    poolmat_dram = nc.inline_tensor(poolmat_np, name="poolmat")
    mask_dram = nc.inline_tensor(mask_np, name="mask")

    # collective bounce buffers (8-core AllToAll: m-eighths <-> channels)
    so_dram = nc.dram_tensor("so_local", [NCORES * ROWS * MQ], bf16)
    a2a_dram = nc.dram_tensor("so_a2a", [NCORES * ROWS * MQ], bf16)

    NT = L // 128        # 32 column tiles of the folded q
    NMM = L // 512       # 8 matmul column chunks

    with TileContext(nc) as tc:
        with (
            tc.tile_pool(name="big", bufs=3) as big_pool,
            tc.tile_pool(name="persist", bufs=1) as persist,
            tc.tile_pool(name="small", bufs=2) as small,
        ):
            with tc.tile_pool(name="psum", bufs=2, space="PSUM") as psum:
                ident = persist.tile([128, 128], f32)
                nc.sync.dma_start(out=ident, in_=ident_dram[:, :])
                poolmat = persist.tile([ROWS, KR], f32)
                nc.sync.dma_start(out=poolmat, in_=poolmat_dram[:, :])

                # ---- kv: pool -> l2-normalise (independent of Q) ----
                kv_sb = big_pool.tile([ROWS, L], f32, tag="kv", bufs=1)
                nc.sync.dma_start(out=kv_sb, in_=kv_in[:, :])
                kn = persist.tile([KR, L], f32)
                for n in range(NMM):
                    ppool = psum.tile([KR, 512], f32, tag="mm512")
                    nc.tensor.matmul(ppool, lhsT=poolmat,
                                     rhs=kv_sb[:, n * 512:(n + 1) * 512],
                                     start=True, stop=True)
                    nc.scalar.copy(kn[:, n * 512:(n + 1) * 512], ppool)
                ksq = small.tile([KR, 1], f32)
                ksc = big_pool.tile([KR, L], f32, tag="sqscr", bufs=1)
                nc.scalar.activation(ksc, kn, Act.Square, accum_out=ksq)
                knrm = small.tile([KR, 1], f32)
                nc.scalar.sqrt(knrm, ksq)
                nc.vector.tensor_scalar_max(knrm, knrm, EPS)
                kinv = small.tile([KR, 1], f32)
                nc.vector.reciprocal(kinv, knrm)
                nc.vector.tensor_scalar_mul(kn, kn, kinv)
                # bf16 copy of normalized kn for the small_out matmul
                kn_bf = persist.tile([KR, L], bf16)
                nc.vector.tensor_copy(kn_bf, kn)

                # ---- full W, transposed: K-chunks of 96, O-blocks of 128 --
                NB = C // 128   # 3 output-row blocks
                NK = GROUP      # 4 channel chunks of 96 (align a2a blocks)
                w_sb = persist.tile([128, NB, C], f32)
                nc.sync.dma_start(
                    out=w_sb,
                    in_=w_in[:, :].rearrange("(b p) c -> p b c", p=128))
                wT = persist.tile([ROWS, NK, NB, 128], bf16)
                for kc in range(NK):
                    for ob in range(NB):
                        pw = psum.tile([ROWS, 128], f32, tag="tp")
                        nc.tensor.transpose(
                            pw, w_sb[:, ob, kc * ROWS:(kc + 1) * ROWS],
                            ident)
                        nc.scalar.copy(wT[:, kc, ob, :], pw)

                # transpose kn -> knT [128, NT, KR]
                knT = persist.tile([128, NT, KR], f32)
                for t in range(NT):
                    pt = psum.tile([128, KR], f32, tag="tp")
                    nc.tensor.transpose(pt, kn[:, t * 128:(t + 1) * 128],
                                        ident[:KR, :KR])
                    nc.scalar.copy(knT[:, t, :], pt)

                # ---- Q: fold 16 chunks + sum of squares ----
                acc = persist.tile([ROWS, L], f32)
                sqparts = persist.tile([ROWS, J], f32)
                for j in range(J):
                    chunk = big_pool.tile([ROWS, L], f32, tag="chunk", bufs=4)
                    nc.sync.dma_start(out=chunk[:, :L // 2],
                                      in_=q_in[:, j, :L // 2])
                    nc.sync.dma_start(out=chunk[:, L // 2:],
                                      in_=q_in[:, j, L // 2:])
                    if j == 0:
                        nc.vector.tensor_copy(acc, chunk)
                    else:
                        nc.vector.tensor_add(acc, acc, chunk)
                    sqscr = big_pool.tile([ROWS, L], f32, tag="sqscr", bufs=1)
                    nc.scalar.activation(sqscr, chunk, Act.Square,
                                         accum_out=sqparts[:, j:j + 1])

                sumsq = small.tile([ROWS, 1], f32)
                nc.vector.reduce_sum(sumsq, sqparts, axis=mybir.AxisListType.X)
                qnrm = small.tile([ROWS, 1], f32)
                nc.scalar.sqrt(qnrm, sumsq)
                nc.vector.tensor_scalar_max(qnrm, qnrm, EPS)
                qinv = small.tile([ROWS, 1], f32)
                nc.vector.reciprocal(qinv, qnrm)

                # transpose acc -> qfT [128, NT, ROWS]
                qfT = persist.tile([128, NT, ROWS], f32)
                for t in range(NT):
                    ptq = psum.tile([128, ROWS], f32, tag="tp")
                    nc.tensor.transpose(ptq, acc[:, t * 128:(t + 1) * 128],
                                        ident[:ROWS, :ROWS])
                    nc.scalar.copy(qfT[:, t, :], ptq)

                # ---- attention logits for both heads in one matmul chain ---
                # out[96, 24]: block diag [48x12 | 48x12] is valid, rest junk
                pattn = psum.tile([ROWS, KR], f32, tag="attn", bufs=1)
                for t in range(NT):
                    nc.tensor.matmul(pattn, lhsT=qfT[:, t, :], rhs=knT[:, t, :],
                                     start=(t == 0), stop=(t == NT - 1))

                # ---- softmax (no max-subtraction: |logits| <= 4) ----
                # Scale+exp the full [96, 24] (off-block junk is bounded),
                # then mask block-diagonally while row-summing in one DVE op.
                mask_sb = persist.tile([ROWS, KR], f32)
                nc.sync.dma_start(out=mask_sb, in_=mask_dram[:, :])
                e_sb = small.tile([ROWS, KR], f32)
                nc.vector.tensor_scalar(e_sb, pattn, qinv, None, Alu.mult)
                nc.scalar.activation(e_sb, e_sb, Act.Exp)
                p_sb = small.tile([ROWS, KR], f32)
                nc.vector.tensor_mul(p_sb, e_sb, mask_sb)
                esum = small.tile([ROWS, 1], f32)
                nc.vector.reduce_sum(esum, p_sb, axis=mybir.AxisListType.X)
                einv = small.tile([ROWS, 1], f32)
                nc.vector.reciprocal(einv, esum)

                # one PE transpose yields block-diagonal pT [24, 96]
                pT = small.tile([KR, ROWS], bf16)
                ptp = psum.tile([KR, ROWS], f32, tag="tp")
                nc.tensor.transpose(ptp, p_sb, ident[:ROWS, :ROWS])
                nc.scalar.copy(pT, ptp)

                # ---- small_out = softmax(p) @ kn (both heads at once) ----
                # the 1/sum(exp) row scale is applied on the PSUM->SBUF copy
                so_sb = persist.tile([ROWS, L], bf16)
                for n in range(NMM):
                    pso = psum.tile([ROWS, 512], f32, tag="mm512")
                    nc.tensor.matmul(pso, lhsT=pT,
                                     rhs=kn_bf[:, n * 512:(n + 1) * 512],
                                     start=True, stop=True)
                    nc.scalar.activation(so_sb[:, n * 512:(n + 1) * 512], pso,
                                         Act.Copy, scale=einv)

                # ---- 8-core AllToAll: shard r = so[:, 512r:512r+512] ----
                # received block g = [96 channel rows of batch g//4,
                # channel block g%4, my m-eighth]
                nc.sync.dma_start(
                    out=so_dram[:].rearrange("(g p m) -> p g m",
                                             g=NCORES, p=ROWS),
                    in_=so_sb.rearrange("p (g m) -> p g m", g=NCORES))
                nc.gpsimd.collective_compute(
                    "AllToAll", Alu.bypass,
                    replica_groups=[[0, 1, 2, 3, 4, 5, 6, 7]],
                    ins=[so_dram[:]],
                    outs=[a2a_dram[:]],
                )

            # ---- projection: y[b, :, my m-eighth] = W @ so_all[b] ----
            # first PSUM pool released; 6 banks as accumulators
            with tc.tile_pool(name="psum_y", bufs=1, space="PSUM") as psum_y:
                a2a_ap = a2a_dram[:].rearrange("(g p m) -> g p m",
                                               g=NCORES, p=ROWS)
                py = [[psum_y.tile([128, MQ], f32, tag=f"y{b}{ob}",
                                   name=f"py{b}{ob}") for ob in range(NB)]
                      for b in range(B)]
                for b in range(B):
                    for kc in range(NK):
                        gt = big_pool.tile([ROWS, MQ], bf16, tag="gath",
                                           bufs=2)
                        nc.sync.dma_start(out=gt,
                                          in_=a2a_ap[GROUP * b + kc, :, :])
                        for ob in range(NB):
                            nc.tensor.matmul(
                                py[b][ob], lhsT=wT[:, kc, ob, :], rhs=gt,
                                start=(kc == 0), stop=(kc == NK - 1))
                for b in range(B):
                    y_ap = y_out[b, :, :].rearrange("(ob p) m -> p ob m",
                                                    p=128)
                    for ob in range(NB):
                        y_sb = small.tile([128, MQ], f32, tag="ysb")
                        nc.scalar.copy(y_sb, py[b][ob])
                        nc.sync.dma_start(out=y_ap[:, ob, :], in_=y_sb)

    if not nc.is_finalized():
        nc.finalize()
    return nc


def _get_nc():
    if "nc" not in _CACHE:
        _CACHE["nc"] = _build()
    return _CACHE["nc"]


def kernel(Q, kv, W_proj, _trace=False):
    from concourse.bass_utils import run_bass_kernel_spmd

    Q = np.ascontiguousarray(Q, dtype=np.float32)
    kv = np.ascontiguousarray(kv, dtype=np.float32)
    W_proj = np.ascontiguousarray(W_proj, dtype=np.float32)

    Qr = Q.reshape(B * C, J, L)
    in_maps = []
    for i in range(NCORES):
        b, a = divmod(i, GROUP)
        sl = slice(96 * a, 96 * a + 96)
        in_maps.append({
            "q": np.ascontiguousarray(Qr[b * C + 96 * a: b * C + 96 * a + 96]),
            "kv": np.ascontiguousarray(kv[b, sl]),
            "w": W_proj,
        })

    nc = _get_nc()
    res = run_bass_kernel_spmd(nc, in_maps, core_ids=list(range(NCORES)),
                               trace=_trace)
    _CACHE["last_results"] = res

    MQ = L // NCORES
    y_small = np.empty((B, C, L), np.float32)
    for i in range(NCORES):
        y_small[:, :, MQ * i: MQ * (i + 1)] = res.results[i]["y"]

    out = np.broadcast_to(y_small[:, :, None, :], (B, C, J, L))
    return np.ascontiguousarray(out).reshape(B, C, H, W)
